# revision 20
# baseline (speedup 1.0000x reference)
"""AdaptiveGaussianConvLayer Trainium2 kernel (8 NeuronCores, SPMD, no collectives).

Math: out[b, j, d] = sum_i V[b, i, d] * W[b, i, j],
      W[b, i, j] = exp(-0.5 * ((j - i - mu[b,i]) / sigma[b,i])^2)
with B=4, N=4096, D=512; sigma in (0.5, 2.5), mu ~ 3*N(0,1).

W underflows to exactly 0.0 in fp32 once |j - i - mu|/sigma >= ~13.2, i.e. for
|j - i| >= ~48.  Each 128-wide j-tile therefore only needs the 3 aligned
128-row i-slabs centered on it (band 384 covers |j-i| <= 128); the rest is
exactly zero, so the banded result matches the dense reference to fp32
rounding.

Sharding: 8 cores = (batch b) x (j-half h).  Core c computes
out[b, h*2048:(h+1)*2048, :].  Host pads V/sigma/mu with 128 zero rows each
side of the core's i-window so all cores run one identical SPMD program
(pad rows have V=0 and contribute nothing).  V is pre-cast to bf16 on the
host (the matmul compute dtype), halving its DMA traffic.

Per-core dataflow (i on partitions, j/d on the free axis):
  z2_s  = Square(r_s * iota + b0_s)     per slab s  (ACT, per-partition scale
                                         r=1/sigma, bias b0=(-128-p-mu)*r)
  W     = Exp(-0.5 * z2)                multi-slab chunks (ACT, bf16 out)
  psum  = sum_k W[slab t+k] .T @ V[slab t+k]   (TensorE, bf16, k=0..2)
  obuf  <- psum copy (DVE/ACT), DMA out in 2-j-tile pairs
Slab s covers local j in [128(s-2), 128(s+2)); with u the iota value,
z = (u - 128 - p - mu)/sigma = (j - i - mu)/sigma.  A few slabs' z2 are
computed on GpSimd (tensor_scalar + square) to offload the ScalarE chain.
"""

import os
import numpy as np
import ml_dtypes

import concourse.bass as bass
import concourse.bacc as bacc
import concourse.mybir as mybir
import concourse.tile as tile
from concourse.bass_utils import run_bass_kernel_spmd

AF = mybir.ActivationFunctionType
ALU = mybir.AluOpType

B, N, D = 4, 4096, 512
NCORES = 8
HALF = N // 2            # 2048 j per core
NSLAB = HALF // 128 + 2  # 18 i-slabs of 128 rows (1 pad slab each side)
VROWS = NSLAB * 128      # 2304
JT = HALF // 128         # 16 j-tiles per core
WWIN = 384               # j-window width per slab

# (start_col, width) of the genuinely used j-window per slab: edge slabs
# serve fewer j-tiles.  Slab s serves j-tiles t in {s-2, s-1, s} & [0, JT).
def _slab_win(s):
    t_lo, t_hi = max(s - 2, 0), min(s, JT - 1)
    lo = (t_lo - (s - 2)) * 128
    return lo, (t_hi - t_lo + 1) * 128

# Exp/unlock chunks; small final chunks shorten the post-ACT tail
CHUNKS = [(0, 3), (3, 3), (6, 3), (9, 3), (12, 3), (15, 1), (16, 1), (17, 1)]

# slabs whose z2 is computed on GpSimd instead of ACT (offload); these are
# emitted first so GpSimd runs ahead of the ACT chain rather than stalling it
GPS_SLABS = (3, 6, 9, 12, 14, 16)

DT_MM = "bf16"
PACK64 = os.environ.get("AGC_PACK64", "1") == "1"
WARMUP = int(os.environ.get("AGC_WARMUP", "8"))
FLATBAR = os.environ.get("AGC_FLATBAR", "1") == "1"

_cached = {}


def _flat_start_barrier(self, *, sem_only=False):
    """Flat all-engine barrier: every engine incs one sem and waits for the
    full count.  One cross-engine hop instead of the ~8-hop sequential chain
    of the stock butterfly (~700ns per cold hop)."""
    arrive = self.alloc_semaphore("flat_barrier_arrive")
    n = len(self.engines)
    for eng in self.engines.values():
        eng.sem_inc(arrive, 1)
    for eng in self.engines.values():
        eng.wait_ge(arrive, n)
    if not hasattr(self, "_flat_barrier_sems"):
        self._flat_barrier_sems = []
    self._flat_barrier_sems.append(arrive)


def _lean_drain_and_barrier(self, tick_clock, wait_clock):
    """Tail trimmed: SP waits for all DMA completion sems, every engine
    drains and incs an arrival sem once, GpSimd waits for the arrivals and
    runs the semaphore clears.  No release phase and no second barrier:
    nrt_execute only returns when every engine's stream is done, and the next
    execution starts fresh, so the clears cannot race anything."""
    from concourse.vector_clock import ScopedClock

    nc = self.nc
    drain_inst = nc.sync.drain()
    wait_clock.add_sem_waits(
        drain_inst.ins, ScopedClock({None: tick_clock.global_clock})
    )
    arrive = nc.alloc_semaphore("tail_arrive")
    n_other = 0
    for eng in nc.engines.values():
        if eng is nc.gpsimd:
            continue
        eng.drain()
        eng.sem_inc(arrive, 1)
        n_other += 1
    nc.gpsimd.drain()
    nc.gpsimd.wait_ge(arrive, n_other)
    popped = nc._tile_sem_poison_stack.pop()
    assert popped is self._sem_poison
    sems = list(self.sems.allocated().values()) + [arrive]
    sems += getattr(nc, "_flat_barrier_sems", [])
    nc.clear_and_free_semaphores(sems)


def build_nc():
    tile.TileContext._drain_and_barrier = _lean_drain_and_barrier
    if FLATBAR:
        bass.Bass.all_engine_barrier = _flat_start_barrier
    f32 = mybir.dt.float32
    bf16 = mybir.dt.bfloat16
    nc = bacc.Bacc("TRN2", target_bir_lowering=False, debug=False)

    # V is pre-cast to bf16 on the host
    vp_d = nc.dram_tensor("Vp", [VROWS, D], bf16, kind="ExternalInput").ap()
    # cst = [iota(384) | q r pairs (36) | b0 r pairs (36) | zero] per partition
    CW = WWIN + 4 * NSLAB + 1
    cst_d = nc.dram_tensor("cst", [128, CW], f32, kind="ExternalInput").ap()
    out_d = nc.dram_tensor("out", [HALF, D], f32, kind="ExternalOutput").ap()

    with tile.TileContext(nc) as tc:
        with (
            tc.tile_pool(name="const", bufs=1) as constp,
            tc.tile_pool(name="big", bufs=1) as bigp,
            tc.tile_pool(name="ps", bufs=3, space=bass.MemorySpace.PSUM) as pspool,
            tc.tile_pool(name="obuf", bufs=3) as opool,
        ):
            cst_t = constp.tile([128, CW], f32)
            nc.sync.dma_start(cst_t[:], cst_d[:])
            iota_t = cst_t[:, 0:WWIN]
            qr = lambda s: (cst_t[:, WWIN + 2 * s : WWIN + 2 * s + 1],
                            cst_t[:, WWIN + 2 * s + 1 : WWIN + 2 * s + 2])
            b0r = lambda s: (cst_t[:, WWIN + 2 * NSLAB + 2 * s : WWIN + 2 * NSLAB + 2 * s + 1],
                             cst_t[:, WWIN + 2 * NSLAB + 2 * s + 1 : WWIN + 2 * NSLAB + 2 * s + 2])
            zero = cst_t[:, CW - 1 : CW]

            vbuf = bigp.tile([128, NSLAB * D], bf16)      # V slabs (bf16)
            zbuf = bigp.tile([128, NSLAB * WWIN], f32)    # z (GpSimd slabs only)
            z2buf = bigp.tile([128, NSLAB * WWIN], f32)   # z^2
            wbuf = bigp.tile([128, NSLAB * WWIN], bf16)   # W

            # PE warm-up: dependency-free matmuls on scratch data so the HAM
            # clock gate reaches 2.4 GHz before the real matmuls start
            wscr = bigp.tile([128, 128 + D], bf16)
            nc.gpsimd.memset(wscr[:], 0.0)
            # zero the never-written z2 edge gaps so chunked Exp reads are
            # fully initialized (values unused by any matmul)
            for s, lo, hi in ((0, 0, 256), (1, 0, 128), (16, 256, 384), (17, 128, 384)):
                nc.gpsimd.memset(z2buf[:, s * WWIN + lo : s * WWIN + hi], 0.0)
            wps = pspool.tile([128, 2 * D], f32, tag="ps")
            for _ in range(WARMUP):
                nc.tensor.matmul(wps[:, 0:D], wscr[:, 0:128], wscr[:, 128:],
                                 start=True, stop=True)

            # V loads on the ACT HWDGE ring (cst + outputs own the SP ring);
            # the issue cost overlaps ACT's wait for cst anyway
            vp3 = vp_d.rearrange("(s p) d -> p s d", p=128)
            vb3 = vbuf[:].rearrange("p (s d) -> p s d", d=D)
            for c in range(3):
                nc.scalar.dma_start(vb3[:, 6 * c : 6 * (c + 1), :],
                                    vp3[:, 6 * c : 6 * (c + 1), :])

            # z2 per slab: ACT Square(r*iota + b0), or GpSimd z=(iota+q)*r, z*z
            def emit_z2(s):
                lo, w = _slab_win(s)
                z2 = z2buf[:, s * WWIN + lo : s * WWIN + lo + w]
                src = iota_t[:, lo : lo + w]
                if s in GPS_SLABS:
                    q, r = qr(s)
                    z = zbuf[:, s * WWIN + lo : s * WWIN + lo + w]
                    nc.gpsimd.tensor_scalar(z, src, q, r, ALU.add, ALU.mult)
                    nc.gpsimd.tensor_tensor(z2, z, z, ALU.mult)
                else:
                    b0, r = b0r(s)
                    nc.scalar.activation(z2, src, AF.Square, bias=b0, scale=r)

            # Exp chunk -> W (const scale/bias, mergeable across slabs)
            def emit_exp(ci):
                s0, ns = CHUNKS[ci]
                lo = s0 * WWIN + _slab_win(s0)[0]
                last = s0 + ns - 1
                hi = last * WWIN + sum(_slab_win(last))
                nc.scalar.activation(wbuf[:, lo:hi], z2buf[:, lo:hi],
                                     AF.Exp, bias=zero, scale=-0.5)

            out3 = out_d.rearrange("(P h p) d -> P p h d", h=2, p=128)

            def emit_jtile(t, ps):
                """3 slabs -> 2 PE slots: full-K matmul on the middle slab,
                then the two boundary slabs need only 64 rows each (band
                |j-i| < 65 covers the true ~48 band) packed into disjoint
                row-groups of the array so they run concurrently."""
                half = t % 2
                out = ps[:, half * D : (half + 1) * D]
                if not PACK64:
                    for k in range(3):
                        ls = t + k
                        nc.tensor.matmul(
                            out,
                            wbuf[:, ls * WWIN + (2 - k) * 128 : ls * WWIN + (3 - k) * 128],
                            vbuf[:, ls * D : (ls + 1) * D],
                            start=(k == 0), stop=(k == 2),
                        )
                    return
                # middle slab t+1: full K=128 (W cols [128,256) of its window)
                ls = t + 1
                nc.tensor.matmul(
                    out,
                    wbuf[:, ls * WWIN + 128 : ls * WWIN + 256],
                    vbuf[:, ls * D : (ls + 1) * D],
                    start=True, stop=False,
                )
                # slab t rows [64,128): i in [128t-64, 128t), W cols [256,384)
                ls = t
                nc.tensor.matmul(
                    out,
                    wbuf[64:128, ls * WWIN + 256 : ls * WWIN + 384],
                    vbuf[64:128, ls * D : (ls + 1) * D],
                    start=False, stop=False,
                    tile_position=(64, 0),
                )
                # slab t+2 rows [0,64): i in [128(t+1), 128(t+1)+64), W cols [0,128)
                ls = t + 2
                nc.tensor.matmul(
                    out,
                    wbuf[0:64, ls * WWIN : ls * WWIN + 128],
                    vbuf[0:64, ls * D : (ls + 1) * D],
                    start=False, stop=True,
                    tile_position=(0, 0),
                )

            # pipeline emission: GpSimd z2 first (runs ahead), then per-chunk
            # ACT z2 -> Exp -> j-tiles as they unlock
            for s in GPS_SLABS:
                emit_z2(s)
            next_t = 0
            ps = ob = None
            for ci, (s0, ns) in enumerate(CHUNKS):
                for s in range(s0, s0 + ns):
                    if s not in GPS_SLABS:
                        emit_z2(s)
                emit_exp(ci)
                # j-tile t needs W of slabs t..t+2  ->  t <= s0+ns-3
                while next_t < JT and next_t <= s0 + ns - 3:
                    t = next_t
                    if t % 2 == 0:
                        ps = pspool.tile([128, 2 * D], f32)
                        ob = opool.tile([128, 2 * D], f32)
                    emit_jtile(t, ps)
                    nc.vector.tensor_copy(ob[:, (t % 2) * D : (t % 2 + 1) * D],
                                          ps[:, (t % 2) * D : (t % 2 + 1) * D])
                    if t % 2 == 1:
                        nc.sync.dma_start(
                            out3[t // 2], ob[:].rearrange("p (h d) -> p h d", h=2))
                    next_t += 1
            assert next_t == JT

    nc.compile()
    return nc


def _get_nc():
    if "nc" not in _cached:
        _cached["nc"] = build_nc()
    return _cached["nc"]


def make_in_maps(V, sigma, mu):
    """Host-side sharding: per-core padded bf16 V rows + scale table."""
    V = np.asarray(V, dtype=np.float32)
    sigma = np.asarray(sigma, dtype=np.float32).reshape(B, N)
    mu = np.asarray(mu, dtype=np.float32).reshape(B, N)
    CW = WWIN + 4 * NSLAB + 1
    pidx = (np.arange(VROWS) % 128).astype(np.float32)
    in_maps = []
    for c in range(NCORES):
        b, h = divmod(c, 2)
        jb = h * HALF
        lo, hi = jb - 128, jb + HALF + 128
        slo, shi = max(lo, 0), min(hi, N)
        vp = np.zeros((VROWS, D), ml_dtypes.bfloat16)
        sig = np.ones(VROWS, np.float32)
        muv = np.zeros(VROWS, np.float32)
        vp[slo - lo : shi - lo] = V[b, slo:shi].astype(ml_dtypes.bfloat16)
        sig[slo - lo : shi - lo] = sigma[b, slo:shi]
        muv[slo - lo : shi - lo] = mu[b, slo:shi]
        r = (np.float32(1.0) / sig).astype(np.float32)
        q = (np.float32(-128.0) - pidx - muv).astype(np.float32)
        b0 = (q * r).astype(np.float32)
        cst = np.zeros((128, CW), np.float32)
        cst[:, 0:WWIN] = np.arange(WWIN, dtype=np.float32)[None, :]
        cst[:, WWIN : WWIN + 2 * NSLAB : 2] = q.reshape(NSLAB, 128).T
        cst[:, WWIN + 1 : WWIN + 2 * NSLAB : 2] = r.reshape(NSLAB, 128).T
        cst[:, WWIN + 2 * NSLAB : WWIN + 4 * NSLAB : 2] = b0.reshape(NSLAB, 128).T
        cst[:, WWIN + 2 * NSLAB + 1 : WWIN + 4 * NSLAB : 2] = r.reshape(NSLAB, 128).T
        in_maps.append({"Vp": vp, "cst": cst})
    return in_maps


def gather(results):
    out = np.empty((B, N, D), np.float32)
    for c in range(NCORES):
        b, h = divmod(c, 2)
        out[b, h * HALF : (h + 1) * HALF] = np.asarray(results[c]["out"])
    return out


def kernel(V, sigma, mu):
    nc = _get_nc()
    in_maps = make_in_maps(V, sigma, mu)
    res = run_bass_kernel_spmd(nc, in_maps, core_ids=list(range(NCORES)))
    return gather(res.results)


# revision 24
# speedup vs baseline: 1.1263x; 1.1263x over previous
"""AdaptiveGaussianConvLayer Trainium2 kernel (8 NeuronCores, SPMD, no collectives).

Math: out[b, j, d] = sum_i V[b, i, d] * W[b, i, j],
      W[b, i, j] = exp(-0.5 * ((j - i - mu[b,i]) / sigma[b,i])^2)
with B=4, N=4096, D=512; sigma in (0.5, 2.5), mu ~ 3*N(0,1).

W underflows to exactly 0.0 in fp32 once |j - i - mu|/sigma >= ~13.2, i.e. for
|j - i| >= ~48 (|mu| <= ~15, 13.3*sigma <= ~33).  On a 64-shifted slab grid
(slab s = rows [128s - 64, 128s + 64) of the core's j-range), each 128-wide
j-tile t needs only slabs {t, t+1}, covering i in [128t - 64, 128t + 192) —
a superset of the true +-48 band with 16 rows of margin — so the banded
result matches the dense reference to fp32 rounding.

Sharding: 8 cores = (batch b) x (j-half h).  Core c computes
out[b, h*2048:(h+1)*2048, :].  Host pads V/sigma/mu with 64 zero rows on each
side of the core's i-window so all cores run one identical SPMD program
(pad rows have V=0 and contribute nothing).  V is pre-cast to bf16 on the
host (the matmul compute dtype), halving its DMA traffic.

Per-core dataflow (i on partitions, j/d on the free axis):
  z2_s  = Square(r_s * iota + b0_s)   per slab s (ACT; per-partition scale
                                       r=1/sigma, bias b0=(-64-p-mu)*r), some
                                       slabs on GpSimd as z=(iota+q)*r; z*z
  W     = Exp(-0.5 * z2)              multi-slab chunks (ACT, bf16 out)
  psum  = sum_{k=0,1} W[slab t+k].T @ V[slab t+k]   (TensorE, K=128 bf16)
  obuf  <- psum copy (DVE/ACT), DMA out in 2-j-tile pairs
Slab s covers local j in [128(s-1), 128(s+1)) (window 256); with u the iota
value, z = (u - 64 - p - mu)/sigma = (j - i - mu)/sigma.

A handful of scratch matmuls (gated on the const DMA) warm the PE HAM clock
gate to 2.4 GHz just before the real matmul stream begins.
"""

import os
import numpy as np
import ml_dtypes

import concourse.bass as bass
import concourse.bacc as bacc
import concourse.mybir as mybir
import concourse.tile as tile
from concourse.bass_utils import run_bass_kernel_spmd

AF = mybir.ActivationFunctionType
ALU = mybir.AluOpType

B, N, D = 4, 4096, 512
NCORES = 8
HALF = N // 2             # 2048 j per core
NSLAB = HALF // 128 + 1   # 17 slabs of 128 rows on the 64-shifted grid
VROWS = NSLAB * 128       # 2176
JT = HALF // 128          # 16 j-tiles per core
WWIN = 256                # j-window width per slab

# genuinely used j-window per slab (edge slabs serve one j-tile)
def _slab_win(s):
    t_lo, t_hi = max(s - 1, 0), min(s, JT - 1)
    lo = (t_lo - (s - 1)) * 128
    return lo, (t_hi - t_lo + 1) * 128

# Exp/unlock chunks (slab start, nslabs); j-tile t unlocks once slab t+1 is
# done.  Smaller final chunks shorten the post-ACT tail.
CHUNKS = [(0, 3), (3, 3), (6, 3), (9, 3), (12, 2), (14, 2), (16, 1)]

# slabs whose z2 is computed on GpSimd (tensor_scalar + square) to offload
# the ScalarE chain; emitted first so GpSimd runs ahead
GPS_SLABS = (2, 4, 6, 8, 10, 12, 14)

WARMUP = int(os.environ.get("AGC_WARMUP", "6"))
FLATBAR = os.environ.get("AGC_FLATBAR", "1") == "1"

_cached = {}


def _flat_start_barrier(self, *, sem_only=False):
    """Flat all-engine barrier: every engine incs one sem and waits for the
    full count — one cross-engine hop instead of the stock sequential chain."""
    arrive = self.alloc_semaphore("flat_barrier_arrive")
    n = len(self.engines)
    for eng in self.engines.values():
        eng.sem_inc(arrive, 1)
    for eng in self.engines.values():
        eng.wait_ge(arrive, n)
    if not hasattr(self, "_flat_barrier_sems"):
        self._flat_barrier_sems = []
    self._flat_barrier_sems.append(arrive)


_stock_drain_and_barrier = tile.TileContext._drain_and_barrier


def _tail_drain_and_barrier(self, tick_clock, wait_clock):
    """Stock tail (its barrier instructions order the in-flight DMA completion
    sems ahead of the clears) + clear the flat-start-barrier sem so
    re-execution starts from zero."""
    _stock_drain_and_barrier(self, tick_clock, wait_clock)
    nc = self.nc
    fs = getattr(nc, "_flat_barrier_sems", [])
    if fs:
        nc.clear_and_free_semaphores(fs)
        nc._flat_barrier_sems = []


def build_nc():
    tile.TileContext._drain_and_barrier = _tail_drain_and_barrier
    f32 = mybir.dt.float32
    bf16 = mybir.dt.bfloat16
    orig_barrier = bass.Bass.all_engine_barrier
    if FLATBAR:
        bass.Bass.all_engine_barrier = _flat_start_barrier
    try:
        nc = bacc.Bacc("TRN2", target_bir_lowering=False, debug=False)
    finally:
        bass.Bass.all_engine_barrier = orig_barrier

    # V pre-cast to bf16 on the host
    vp_d = nc.dram_tensor("Vp", [VROWS, D], bf16, kind="ExternalInput").ap()
    # cst = [iota(256) | q r pairs | b0 r pairs | zero] per partition
    CW = WWIN + 4 * NSLAB + 1
    cst_d = nc.dram_tensor("cst", [128, CW], f32, kind="ExternalInput").ap()
    out_d = nc.dram_tensor("out", [HALF, D], f32, kind="ExternalOutput").ap()

    with tile.TileContext(nc) as tc:
        with (
            tc.tile_pool(name="const", bufs=1) as constp,
            tc.tile_pool(name="big", bufs=1) as bigp,
            tc.tile_pool(name="ps", bufs=3, space=bass.MemorySpace.PSUM) as pspool,
            tc.tile_pool(name="obuf", bufs=3) as opool,
        ):
            cst_t = constp.tile([128, CW], f32, name="cst_t")
            # cst on the ACT ring first: ACT is idle until it arrives anyway
            nc.scalar.dma_start(cst_t[:], cst_d[:])
            iota_t = cst_t[:, 0:WWIN]
            qr = lambda s: (cst_t[:, WWIN + 2 * s : WWIN + 2 * s + 1],
                            cst_t[:, WWIN + 2 * s + 1 : WWIN + 2 * s + 2])
            b0r = lambda s: (cst_t[:, WWIN + 2 * NSLAB + 2 * s : WWIN + 2 * NSLAB + 2 * s + 1],
                             cst_t[:, WWIN + 2 * NSLAB + 2 * s + 1 : WWIN + 2 * NSLAB + 2 * s + 2])
            zero = cst_t[:, CW - 1 : CW]

            vbuf = bigp.tile([128, NSLAB * D], bf16, name="vbuf")
            zbuf = bigp.tile([128, NSLAB * WWIN], f32, name="zbuf")
            z2buf = bigp.tile([128, NSLAB * WWIN], f32, name="z2buf")
            wbuf = bigp.tile([128, NSLAB * WWIN], bf16, name="wbuf")

            # PE warm-up: scratch matmuls gated on the cst DMA (zeros x cst
            # bits) so the HAM clock gate is at 2.4 GHz when real MMs start
            wscr = bigp.tile([128, 128], bf16, name="wscr")
            nc.gpsimd.memset(wscr[:], 0.0)
            # zero the never-written z2 edge gaps read by chunked Exp
            nc.gpsimd.memset(z2buf[:, 0:128], 0.0)
            nc.gpsimd.memset(z2buf[:, 16 * WWIN + 128 : 17 * WWIN], 0.0)
            wps = pspool.tile([128, 2 * D], f32, tag="ps", name="wps")
            wrhs = cst_t[:, 0:WWIN].bitcast(bf16)  # [128, 512] bf16 view
            for _ in range(WARMUP):
                nc.tensor.matmul(wps[:, 0:D], wscr[:], wrhs,
                                 start=True, stop=True)

            # V loads: slabs 0-5 on the SP ring, 6-11 / 12-16 on the ACT ring
            vp3 = vp_d.rearrange("(s p) d -> p s d", p=128)
            vb3 = vbuf[:].rearrange("p (s d) -> p s d", d=D)
            for eng, lo, hi in ((nc.sync, 0, 6), (nc.scalar, 6, 12),
                                (nc.scalar, 12, 17)):
                eng.dma_start(vb3[:, lo:hi, :], vp3[:, lo:hi, :])

            # z2 per slab: ACT Square(r*iota + b0), or GpSimd z=(iota+q)*r, z*z
            def emit_z2(s):
                lo, w = _slab_win(s)
                z2 = z2buf[:, s * WWIN + lo : s * WWIN + lo + w]
                src = iota_t[:, lo : lo + w]
                if s in GPS_SLABS:
                    q, r = qr(s)
                    z = zbuf[:, s * WWIN + lo : s * WWIN + lo + w]
                    nc.gpsimd.tensor_scalar(z, src, q, r, ALU.add, ALU.mult)
                    nc.gpsimd.tensor_tensor(z2, z, z, ALU.mult)
                else:
                    b0, r = b0r(s)
                    nc.scalar.activation(z2, src, AF.Square, bias=b0, scale=r)

            def emit_exp(ci):
                s0, ns = CHUNKS[ci]
                lo = s0 * WWIN + _slab_win(s0)[0]
                last = s0 + ns - 1
                hi = last * WWIN + sum(_slab_win(last))
                nc.scalar.activation(wbuf[:, lo:hi], z2buf[:, lo:hi],
                                     AF.Exp, bias=zero, scale=-0.5)

            out3 = out_d.rearrange("(P h p) d -> P p h d", h=2, p=128)

            def emit_jtile(t, ps):
                half = t % 2
                out = ps[:, half * D : (half + 1) * D]
                for k in range(2):
                    ls = t + k
                    nc.tensor.matmul(
                        out,
                        wbuf[:, ls * WWIN + (1 - k) * 128 : ls * WWIN + (2 - k) * 128],
                        vbuf[:, ls * D : (ls + 1) * D],
                        start=(k == 0),
                        stop=(k == 1),
                    )

            # pipeline emission: GpSimd z2 first (runs ahead), then per-chunk
            # ACT z2 -> Exp -> j-tiles as they unlock
            for s in GPS_SLABS:
                emit_z2(s)
            next_t = 0
            ps = ob = None
            for ci, (s0, ns) in enumerate(CHUNKS):
                for s in range(s0, s0 + ns):
                    if s not in GPS_SLABS:
                        emit_z2(s)
                emit_exp(ci)
                # j-tile t needs W of slabs t, t+1  ->  t <= s0+ns-2
                while next_t < JT and next_t <= s0 + ns - 2:
                    t = next_t
                    if t % 2 == 0:
                        ps = pspool.tile([128, 2 * D], f32, name="ps")
                        ob = opool.tile([128, 2 * D], f32, name="ob")
                    emit_jtile(t, ps)
                    eng = nc.scalar if t % 8 == 5 else nc.vector
                    if eng is nc.scalar:
                        nc.scalar.activation(
                            ob[:, (t % 2) * D : (t % 2 + 1) * D],
                            ps[:, (t % 2) * D : (t % 2 + 1) * D], AF.Copy)
                    else:
                        nc.vector.tensor_copy(
                            ob[:, (t % 2) * D : (t % 2 + 1) * D],
                            ps[:, (t % 2) * D : (t % 2 + 1) * D])
                    if t % 2 == 1:
                        nc.sync.dma_start(
                            out3[t // 2], ob[:].rearrange("p (h d) -> p h d", h=2))
                    next_t += 1
            assert next_t == JT

    nc.compile()
    return nc


def _get_nc():
    if "nc" not in _cached:
        _cached["nc"] = build_nc()
    return _cached["nc"]


def make_in_maps(V, sigma, mu):
    """Host-side sharding: per-core padded bf16 V rows + scale table."""
    V = np.asarray(V, dtype=np.float32)
    sigma = np.asarray(sigma, dtype=np.float32).reshape(B, N)
    mu = np.asarray(mu, dtype=np.float32).reshape(B, N)
    CW = WWIN + 4 * NSLAB + 1
    pidx = (np.arange(VROWS) % 128).astype(np.float32)
    in_maps = []
    for c in range(NCORES):
        b, h = divmod(c, 2)
        jb = h * HALF
        lo, hi = jb - 64, jb + HALF + 64
        slo, shi = max(lo, 0), min(hi, N)
        vp = np.zeros((VROWS, D), ml_dtypes.bfloat16)
        sig = np.ones(VROWS, np.float32)
        muv = np.zeros(VROWS, np.float32)
        vp[slo - lo : shi - lo] = V[b, slo:shi].astype(ml_dtypes.bfloat16)
        sig[slo - lo : shi - lo] = sigma[b, slo:shi]
        muv[slo - lo : shi - lo] = mu[b, slo:shi]
        r = (np.float32(1.0) / sig).astype(np.float32)
        q = (np.float32(-64.0) - pidx - muv).astype(np.float32)
        b0 = (q * r).astype(np.float32)
        cst = np.zeros((128, CW), np.float32)
        cst[:, 0:WWIN] = np.arange(WWIN, dtype=np.float32)[None, :]
        cst[:, WWIN : WWIN + 2 * NSLAB : 2] = q.reshape(NSLAB, 128).T
        cst[:, WWIN + 1 : WWIN + 2 * NSLAB : 2] = r.reshape(NSLAB, 128).T
        cst[:, WWIN + 2 * NSLAB : WWIN + 4 * NSLAB : 2] = b0.reshape(NSLAB, 128).T
        cst[:, WWIN + 2 * NSLAB + 1 : WWIN + 4 * NSLAB : 2] = r.reshape(NSLAB, 128).T
        in_maps.append({"Vp": vp, "cst": cst})
    return in_maps


def gather(results):
    out = np.empty((B, N, D), np.float32)
    for c in range(NCORES):
        b, h = divmod(c, 2)
        out[b, h * HALF : (h + 1) * HALF] = np.asarray(results[c]["out"])
    return out


def kernel(V, sigma, mu):
    nc = _get_nc()
    in_maps = make_in_maps(V, sigma, mu)
    res = run_bass_kernel_spmd(nc, in_maps, core_ids=list(range(NCORES)))
    return gather(res.results)
